# revision 18
# baseline (speedup 1.0000x reference)
"""ChannelMHSA on Trainium2 (Bass/Tile), data-parallel over batch on 8 cores.

Reference computation (per batch b of x [N, C]):
    qkv  = x @ w_qkv                      # [N, 3C], columns ordered (s, h, d)
    q, k, v per head h: [N, D]
    z_h  = k_h^T @ v_h / sqrt(D)          # [D, D]
    A_h  = softmax(z_h, axis=-1)
    T_h  = A_h @ q_h^T                    # [D, N]
    out[n, h*D+d] = T_h[d, n]
    y    = out @ w_out                    # [N, C]

b_qkv / b_out are all-zero by construction (see input spec) and are ignored.

Kernel layout choices per core (BS=4 batches):
  - xT [C, N] built by PE transposes (6x8 [128,128] blocks per batch).
  - kv = x @ w_qkv[:, C:3C] computed N-major (lhsT = xT chunks).
  - qT = w_q^T @ x^T computed C-major directly (lhsT = w_q chunks,
    rhs = xT chunks), so q never needs a separate transpose.
  - z per head with rhs packed 4 heads wide (free=256) for PE efficiency.
  - A^T placed into a block-diagonal [128,128] lhsT per head pair so
    T for two heads is one K=128 matmul per 512 output columns.
  - y = out @ w_out with lhsT = outT chunks.
"""

import os
import sys
from contextlib import ExitStack

import numpy as np

for _p in ("/opt/trn_rl_repo", "/opt/pypackages"):
    if _p not in sys.path:
        sys.path.append(_p)

import concourse.bacc as bacc
import concourse.mybir as mybir
import concourse.tile as tile
from concourse import bass_utils, masks

B, N, C = 32, 1024, 768
H, D = 12, 64
P = 128
NCORES = 8
BS = B // NCORES          # batches per core
KC = C // P               # 6 contraction chunks over C
NM = N // P               # 8 chunks over N
F32 = mybir.dt.float32
F32R = mybir.dt.float32r

# float32r runs the PE at 4x fp32 speed for free-dim >= 256 with slightly
# reduced mantissa precision. Override with BASS_MM_DT=f32 to compare.
MM_DT_NAME = os.environ.get("BASS_MM_DT", "f32r")


def _emit(ctx, tc, mm_dt, x_d, wqkv_d, wo_d, y_d):
    nc = tc.nc

    mdt = mm_dt          # dtype for tiles consumed by regular matmuls
    def wcast(ap):       # DRAM-side view for weight DMAs
        return ap.bitcast(mdt) if mdt is not F32 else ap

    const = ctx.enter_context(tc.tile_pool(name="const", bufs=1))
    xin_pool = ctx.enter_context(tc.tile_pool(name="xin", bufs=2))
    xt_pool = ctx.enter_context(tc.tile_pool(name="xtp", bufs=6))
    kv_pool = ctx.enter_context(tc.tile_pool(name="kvp", bufs=8))
    # qT and outT share slots: outT[pr] is produced right after the T matmul
    # of pair pr, which is also the last reader of qT[pr] - zero stall.
    qt_pool = ctx.enter_context(tc.tile_pool(name="qtp", bufs=6))
    y_pool = ctx.enter_context(tc.tile_pool(name="yp", bufs=2))
    sm_pool = ctx.enter_context(tc.tile_pool(name="smp", bufs=4))
    psA = ctx.enter_context(tc.tile_pool(name="psA", bufs=3, space="PSUM"))
    psB = ctx.enter_context(tc.tile_pool(name="psB", bufs=3, space="PSUM"))
    psZ = ctx.enter_context(tc.tile_pool(name="psZ", bufs=2, space="PSUM"))

    ident = const.tile([P, P], F32, tag="ident", name="ident")
    masks.make_identity(nc, ident[:])

    # Two persistent block-diag lhsT tiles for the T matmul, zeroed once via
    # a rounding copy (memset cannot produce float32r). Only the diagonal
    # blocks are rewritten afterwards, so the off-diag zeros persist.
    zeros = const.tile([P, P], F32, tag="zeros", name="zeros")
    nc.vector.memset(zeros[:], 0.0)
    a2_tiles = []
    for i in range(2):
        a2t = const.tile([P, P], mdt, tag=f"a2_{i}", name=f"a2_{i}")
        nc.vector.tensor_copy(a2t[:], zeros[:])
        a2_tiles.append(a2t)

    # Weight loads go on the Activation HWDGE queue so they don't delay the
    # first x chunks (Sync queue) that gate the first PE transposes. Order
    # matters: wq first (gates the qT phase, which is emitted before kv),
    # then wkv, then wo.
    wq = []
    for p in range(KC):
        t = const.tile([P, C], mdt, tag=f"wq{p}", name=f"wq{p}")
        nc.scalar.dma_start(t[:], wcast(wqkv_d[p * P:(p + 1) * P, 0:C]))
        wq.append(t)
    wkv = []
    for p in range(KC):
        t = const.tile([P, 2 * C], mdt, tag=f"wkv{p}", name=f"wkv{p}")
        nc.scalar.dma_start(t[:], wcast(wqkv_d[p * P:(p + 1) * P, C:3 * C]))
        wkv.append(t)
    wo = []
    for p in range(KC):
        t = const.tile([P, C], mdt, tag=f"wo{p}", name=f"wo{p}")
        nc.scalar.dma_start(t[:], wcast(wo_d[p * P:(p + 1) * P, :]))
        wo.append(t)

    for b in range(BS):
        # ---- Phase A: load x, transpose to xT [C, N] ----
        xT = [xt_pool.tile([P, N], mdt, tag="xT", name=f"xT{b}_{p}")
              for p in range(KC)]
        for m in range(NM):
            xin = xin_pool.tile([P, C], F32, tag="xin", name=f"xin{b}_{m}")
            nc.sync.dma_start(xin[:], x_d[b, m * P:(m + 1) * P, :])
            for p in range(KC):
                tp = psA.tile([P, P], F32, tag="tp", name=f"tpx{b}_{m}_{p}",
                              space="PSUM")
                nc.tensor.transpose(tp[:], xin[:, p * P:(p + 1) * P], ident[:])
                nc.vector.tensor_copy(xT[p][:, m * P:(m + 1) * P], tp[:])

        # ---- Phase B1: qT = w_q^T @ x^T, C-major (w_q lands first) ----
        qT = []
        for po in range(KC):
            qtt = qt_pool.tile([P, N], mdt, tag="qT", name=f"qT{b}_{po}")
            qT.append(qtt)
            for nf in range(2):
                ps = psB.tile([P, 512], F32, tag="psB", name=f"psqt{b}_{po}_{nf}",
                              space="PSUM")
                for p in range(KC):
                    nc.tensor.matmul(
                        ps[:],
                        wq[p][:, po * P:(po + 1) * P],
                        xT[p][:, nf * 512:(nf + 1) * 512],
                        start=(p == 0), stop=(p == KC - 1))
                nc.vector.tensor_copy(qtt[:, nf * 512:(nf + 1) * 512], ps[:])

        # ---- Phase B2: kv = x @ w_qkv[:, C:3C], N-major ----
        kv = []
        for m in range(NM):
            kvt = kv_pool.tile([P, 2 * C], mdt, tag="kv", name=f"kv{b}_{m}")
            kv.append(kvt)
            for f in range(3):
                ps = psB.tile([P, 512], F32, tag="psB", name=f"pskv{b}_{m}_{f}",
                              space="PSUM")
                for p in range(KC):
                    nc.tensor.matmul(
                        ps[:],
                        xT[p][:, m * P:(m + 1) * P],
                        wkv[p][:, f * 512:(f + 1) * 512],
                        start=(p == 0), stop=(p == KC - 1))
                nc.vector.tensor_copy(kvt[:, f * 512:(f + 1) * 512], ps[:])

        # ---- Phase C: attention, software-pipelined by one head pair so the
        # next pair's z matmuls fill the PE while this pair's softmax runs on
        # DVE/ACT. ----
        outT = [qt_pool.tile([P, N], mdt, tag="qT", name=f"outT{b}_{p}")
                for p in range(KC)]
        zps_pair = {}
        for step in range(KC + 1):
            if step < KC:
                pr, q4 = step, step // 2
                # z for both heads of the pair in one chain: lhsT packs the
                # two heads' k (M=128), rhs packs 4 heads of v (free=256).
                # Head 2pr lands on psum rows 0:64, head 2pr+1 on 64:128.
                zps = psZ.tile([P, 256], F32, tag="z", name=f"z{b}_{pr}",
                               space="PSUM")
                zps_pair[pr] = zps
                for m in range(NM):
                    nc.tensor.matmul(
                        zps[:],
                        kv[m][:, 2 * pr * D:(2 * pr + 2) * D],
                        kv[m][:, C + q4 * 256:C + (q4 + 1) * 256],
                        start=(m == 0), stop=(m == NM - 1))
            if step == 0:
                continue
            pr = step - 1
            a2 = a2_tiles[pr % 2]
            zps = zps_pair.pop(pr)
            for j in range(2):
                h = 2 * pr + j
                rb = j * D                  # psum row base for this head
                cb = (h % 4) * D
                zsl = zps[rb:rb + D, cb:cb + D]
                # softmax(z / 8) along free dim (partition range rb:rb+64)
                negmax = sm_pool.tile([P, 1], F32, tag="negmax", name=f"nm{b}_{h}")
                nc.vector.reduce_max(negmax[rb:rb + D, :], zsl,
                                     axis=mybir.AxisListType.X, negate=True)
                nmx = sm_pool.tile([P, 1], F32, tag="nmx", name=f"nmx{b}_{h}")
                nc.scalar.mul(nmx[rb:rb + D, :], negmax[rb:rb + D, :], 0.125)
                aex = sm_pool.tile([P, D], F32, tag="aex", name=f"aex{b}_{h}")
                ssum = sm_pool.tile([P, 1], F32, tag="ssum", name=f"ss{b}_{h}")
                nc.scalar.activation(aex[rb:rb + D, :], zsl,
                                     mybir.ActivationFunctionType.Exp,
                                     bias=nmx[rb:rb + D, :], scale=0.125,
                                     accum_out=ssum[rb:rb + D, :])
                rinv = sm_pool.tile([P, 1], F32, tag="rinv", name=f"ri{b}_{h}")
                nc.vector.reciprocal(rinv[rb:rb + D, :], ssum[rb:rb + D, :])
                nc.vector.tensor_scalar_mul(aex[rb:rb + D, :], aex[rb:rb + D, :],
                                            rinv[rb:rb + D, :])
                # A^T into block-diag slot j of a2. The PE can only write
                # transpose outputs at PSUM partition 0, and compute engines
                # cannot shift partitions, so the odd head goes through a
                # small SBUF->SBUF DMA to land on partitions 64:128.
                tp = psA.tile([P, D], F32, tag="tp", name=f"tpa{b}_{h}",
                              space="PSUM")
                nc.tensor.transpose(tp[0:D, 0:D], aex[rb:rb + D, :],
                                    ident[rb:rb + D, rb:rb + D])
                if j == 0:
                    nc.vector.tensor_copy(a2[0:D, 0:D], tp[0:D, 0:D])
                else:
                    at_sb = sm_pool.tile([D, D], mdt, tag="at", name=f"at{b}_{h}")
                    nc.vector.tensor_copy(at_sb[:], tp[0:D, 0:D])
                    nc.sync.dma_start(a2[D:2 * D, D:2 * D], at_sb[:])
            # T for both heads of the pair: one K=128 matmul per 512 cols
            for nf in range(2):
                ps = psB.tile([P, 512], F32, tag="psB", name=f"psT{b}_{pr}_{nf}",
                              space="PSUM")
                nc.tensor.matmul(ps[:], a2[:],
                                 qT[pr][:, nf * 512:(nf + 1) * 512],
                                 start=True, stop=True)
                nc.vector.tensor_copy(outT[pr][:, nf * 512:(nf + 1) * 512],
                                      ps[:])

        # ---- Phase D: y = out @ w_out ----
        for m in range(NM):
            yt = y_pool.tile([P, C], F32, tag="y", name=f"y{b}_{m}")
            for f in range(2):
                ps = psB.tile([P, 384], F32, tag="psB", name=f"psy{b}_{m}_{f}",
                              space="PSUM")
                for p in range(KC):
                    nc.tensor.matmul(
                        ps[:],
                        outT[p][:, m * P:(m + 1) * P],
                        wo[p][:, f * 384:(f + 1) * 384],
                        start=(p == 0), stop=(p == KC - 1))
                nc.vector.tensor_copy(yt[:, f * 384:(f + 1) * 384], ps[:])
            nc.sync.dma_start(y_d[b, m * P:(m + 1) * P, :], yt[:])


_BUILD_CACHE = {}


def build_program(mm_dt_name=MM_DT_NAME):
    if mm_dt_name in _BUILD_CACHE:
        return _BUILD_CACHE[mm_dt_name]
    mm_dt = F32R if mm_dt_name == "f32r" else F32
    nc = bacc.Bacc("TRN2", target_bir_lowering=False, debug=False,
                   num_devices=NCORES)
    x_d = nc.dram_tensor("x", [BS, N, C], F32, kind="ExternalInput").ap()
    wqkv_d = nc.dram_tensor("w_qkv", [C, 3 * C], F32, kind="ExternalInput").ap()
    wo_d = nc.dram_tensor("w_out", [C, C], F32, kind="ExternalInput").ap()
    y_d = nc.dram_tensor("y", [BS, N, C], F32, kind="ExternalOutput").ap()
    with tile.TileContext(nc) as tc:
        with ExitStack() as ctx:
            _emit(ctx, tc, mm_dt, x_d, wqkv_d, wo_d, y_d)
    nc.compile()
    _BUILD_CACHE[mm_dt_name] = nc
    return nc


def make_in_maps(x, w_qkv, w_out):
    x = np.ascontiguousarray(np.asarray(x, dtype=np.float32))
    w_qkv = np.ascontiguousarray(np.asarray(w_qkv, dtype=np.float32))
    w_out = np.ascontiguousarray(np.asarray(w_out, dtype=np.float32))
    return [
        {"x": x[i * BS:(i + 1) * BS], "w_qkv": w_qkv, "w_out": w_out}
        for i in range(NCORES)
    ]


def kernel(x, w_qkv, b_qkv=None, w_out=None, b_out=None, **_unused):
    nc = build_program()
    in_maps = make_in_maps(x, w_qkv, w_out)
    res = bass_utils.run_bass_kernel_spmd(nc, in_maps,
                                          core_ids=list(range(NCORES)))
    y = np.concatenate([res.results[i]["y"] for i in range(NCORES)], axis=0)
    return np.asarray(y, dtype=np.float32)


# revision 19
# speedup vs baseline: 1.0477x; 1.0477x over previous
"""ChannelMHSA on Trainium2 (Bass/Tile), data-parallel over batch on 8 cores.

Reference computation (per batch b of x [N, C]):
    qkv  = x @ w_qkv                      # [N, 3C], columns ordered (s, h, d)
    q, k, v per head h: [N, D]
    z_h  = k_h^T @ v_h / sqrt(D)          # [D, D]
    A_h  = softmax(z_h, axis=-1)
    T_h  = A_h @ q_h^T                    # [D, N]
    out[n, h*D+d] = T_h[d, n]
    y    = out @ w_out                    # [N, C]

b_qkv / b_out are all-zero by construction (see input spec) and are ignored.

Kernel layout choices per core (BS=4 batches):
  - xT [C, N] built by PE transposes (6x8 [128,128] blocks per batch).
  - kv = x @ w_qkv[:, C:3C] computed N-major (lhsT = xT chunks).
  - qT = w_q^T @ x^T computed C-major directly (lhsT = w_q chunks,
    rhs = xT chunks), so q never needs a separate transpose.
  - z per head with rhs packed 4 heads wide (free=256) for PE efficiency.
  - A^T placed into a block-diagonal [128,128] lhsT per head pair so
    T for two heads is one K=128 matmul per 512 output columns.
  - y = out @ w_out with lhsT = outT chunks.
"""

import os
import sys
from contextlib import ExitStack

import numpy as np

for _p in ("/opt/trn_rl_repo", "/opt/pypackages"):
    if _p not in sys.path:
        sys.path.append(_p)

import concourse.bacc as bacc
import concourse.mybir as mybir
import concourse.tile as tile
from concourse import bass_utils, masks

B, N, C = 32, 1024, 768
H, D = 12, 64
P = 128
NCORES = 8
BS = B // NCORES          # batches per core
KC = C // P               # 6 contraction chunks over C
NM = N // P               # 8 chunks over N
F32 = mybir.dt.float32
F32R = mybir.dt.float32r

# float32r runs the PE at 4x fp32 speed for free-dim >= 256 with slightly
# reduced mantissa precision. Override with BASS_MM_DT=f32 to compare.
MM_DT_NAME = os.environ.get("BASS_MM_DT", "f32r")


def _emit(ctx, tc, mm_dt, x_d, wqkv_d, wo_d, y_d):
    nc = tc.nc

    mdt = mm_dt          # dtype for tiles consumed by regular matmuls
    def wcast(ap):       # DRAM-side view for weight DMAs
        return ap.bitcast(mdt) if mdt is not F32 else ap

    const = ctx.enter_context(tc.tile_pool(name="const", bufs=1))
    xin_pool = ctx.enter_context(tc.tile_pool(name="xin", bufs=2))
    xt_pool = ctx.enter_context(tc.tile_pool(name="xtp", bufs=6))
    kv_pool = ctx.enter_context(tc.tile_pool(name="kvp", bufs=8))
    # qT and outT share slots: outT[pr] is produced right after the T matmul
    # of pair pr, which is also the last reader of qT[pr] - zero stall.
    qt_pool = ctx.enter_context(tc.tile_pool(name="qtp", bufs=6))
    y_pool = ctx.enter_context(tc.tile_pool(name="yp", bufs=2))
    sm_pool = ctx.enter_context(tc.tile_pool(name="smp", bufs=4))
    psA = ctx.enter_context(tc.tile_pool(name="psA", bufs=3, space="PSUM"))
    psB = ctx.enter_context(tc.tile_pool(name="psB", bufs=3, space="PSUM"))
    psZ = ctx.enter_context(tc.tile_pool(name="psZ", bufs=2, space="PSUM"))

    ident = const.tile([P, P], F32, tag="ident", name="ident")
    masks.make_identity(nc, ident[:])

    # Two persistent block-diag lhsT tiles for the T matmul, zeroed once via
    # a rounding copy (memset cannot produce float32r). Only the diagonal
    # blocks are rewritten afterwards, so the off-diag zeros persist.
    zeros = const.tile([P, P], F32, tag="zeros", name="zeros")
    nc.vector.memset(zeros[:], 0.0)
    a2_tiles = []
    for i in range(2):
        a2t = const.tile([P, P], mdt, tag=f"a2_{i}", name=f"a2_{i}")
        nc.vector.tensor_copy(a2t[:], zeros[:])
        a2_tiles.append(a2t)

    # Weight loads go on the Activation HWDGE queue so they don't delay the
    # first x chunks (Sync queue) that gate the first PE transposes. Order
    # matters: wq first (gates the qT phase, which is emitted before kv),
    # then wkv, then wo.
    wq = []
    for p in range(KC):
        t = const.tile([P, C], mdt, tag=f"wq{p}", name=f"wq{p}")
        nc.scalar.dma_start(t[:], wcast(wqkv_d[p * P:(p + 1) * P, 0:C]))
        wq.append(t)
    wkv = []
    for p in range(KC):
        t = const.tile([P, 2 * C], mdt, tag=f"wkv{p}", name=f"wkv{p}")
        nc.scalar.dma_start(t[:], wcast(wqkv_d[p * P:(p + 1) * P, C:3 * C]))
        wkv.append(t)
    wo = []
    for p in range(KC):
        t = const.tile([P, C], mdt, tag=f"wo{p}", name=f"wo{p}")
        nc.scalar.dma_start(t[:], wcast(wo_d[p * P:(p + 1) * P, :]))
        wo.append(t)

    for b in range(BS):
        # ---- Phase A: load x, transpose to xT [C, N] ----
        xT = [xt_pool.tile([P, N], mdt, tag="xT", name=f"xT{b}_{p}")
              for p in range(KC)]
        for m in range(NM):
            xin = xin_pool.tile([P, C], F32, tag="xin", name=f"xin{b}_{m}")
            nc.sync.dma_start(xin[:], x_d[b, m * P:(m + 1) * P, :])
            for p in range(KC):
                tp = psA.tile([P, P], F32, tag="tp", name=f"tpx{b}_{m}_{p}",
                              space="PSUM")
                nc.tensor.transpose(tp[:], xin[:, p * P:(p + 1) * P], ident[:])
                nc.vector.tensor_copy(xT[p][:, m * P:(m + 1) * P], tp[:])

        # ---- Phase B1: qT = w_q^T @ x^T, C-major (w_q lands first) ----
        qT = []
        for po in range(KC):
            qtt = qt_pool.tile([P, N], mdt, tag="qT", name=f"qT{b}_{po}")
            qT.append(qtt)
            for nf in range(2):
                ps = psB.tile([P, 512], F32, tag="psB", name=f"psqt{b}_{po}_{nf}",
                              space="PSUM")
                for p in range(KC):
                    nc.tensor.matmul(
                        ps[:],
                        wq[p][:, po * P:(po + 1) * P],
                        xT[p][:, nf * 512:(nf + 1) * 512],
                        start=(p == 0), stop=(p == KC - 1))
                nc.vector.tensor_copy(qtt[:, nf * 512:(nf + 1) * 512], ps[:])

        # ---- Phase B2: kv = x @ w_qkv[:, C:3C], N-major ----
        kv = []
        for m in range(NM):
            kvt = kv_pool.tile([P, 2 * C], mdt, tag="kv", name=f"kv{b}_{m}")
            kv.append(kvt)
            for f in range(3):
                ps = psB.tile([P, 512], F32, tag="psB", name=f"pskv{b}_{m}_{f}",
                              space="PSUM")
                for p in range(KC):
                    nc.tensor.matmul(
                        ps[:],
                        xT[p][:, m * P:(m + 1) * P],
                        wkv[p][:, f * 512:(f + 1) * 512],
                        start=(p == 0), stop=(p == KC - 1))
                nc.vector.tensor_copy(kvt[:, f * 512:(f + 1) * 512], ps[:])

        # ---- Phase C: attention, software-pipelined by one head pair so the
        # next pair's z matmuls fill the PE while this pair's softmax runs on
        # DVE/ACT. ----
        outT = [qt_pool.tile([P, N], mdt, tag="qT", name=f"outT{b}_{p}")
                for p in range(KC)]
        zps_pair = {}
        for step in range(KC + 1):
            if step < KC:
                pr, q4 = step, step // 2
                # z for both heads of the pair in one chain: lhsT packs the
                # two heads' k (M=128), rhs packs 4 heads of v (free=256).
                # Head 2pr lands on psum rows 0:64, head 2pr+1 on 64:128.
                zps = psZ.tile([P, 256], F32, tag="z", name=f"z{b}_{pr}",
                               space="PSUM")
                zps_pair[pr] = zps
                for m in range(NM):
                    nc.tensor.matmul(
                        zps[:],
                        kv[m][:, 2 * pr * D:(2 * pr + 2) * D],
                        kv[m][:, C + q4 * 256:C + (q4 + 1) * 256],
                        start=(m == 0), stop=(m == NM - 1))
            if step == 0:
                continue
            pr = step - 1
            a2 = a2_tiles[pr % 2]
            zps = zps_pair.pop(pr)
            for j in range(2):
                h = 2 * pr + j
                rb = j * D                  # psum row base for this head
                cb = (h % 4) * D
                zsl = zps[rb:rb + D, cb:cb + D]
                # softmax(z / 8) along free dim (partition range rb:rb+64)
                negmax = sm_pool.tile([P, 1], F32, tag="negmax", name=f"nm{b}_{h}")
                nc.vector.reduce_max(negmax[rb:rb + D, :], zsl,
                                     axis=mybir.AxisListType.X, negate=True)
                nmx = sm_pool.tile([P, 1], F32, tag="nmx", name=f"nmx{b}_{h}")
                nc.scalar.mul(nmx[rb:rb + D, :], negmax[rb:rb + D, :], 0.125)
                aex = sm_pool.tile([P, D], F32, tag="aex", name=f"aex{b}_{h}")
                ssum = sm_pool.tile([P, 1], F32, tag="ssum", name=f"ss{b}_{h}")
                nc.scalar.activation(aex[rb:rb + D, :], zsl,
                                     mybir.ActivationFunctionType.Exp,
                                     bias=nmx[rb:rb + D, :], scale=0.125,
                                     accum_out=ssum[rb:rb + D, :])
                rinv = sm_pool.tile([P, 1], F32, tag="rinv", name=f"ri{b}_{h}")
                nc.vector.reciprocal(rinv[rb:rb + D, :], ssum[rb:rb + D, :])
                nc.vector.tensor_scalar_mul(aex[rb:rb + D, :], aex[rb:rb + D, :],
                                            rinv[rb:rb + D, :])
                # A^T into block-diag slot j of a2 via a REGULAR matmul
                # (aex^T @ I). Unlike transpose-mode, a regular matmul may
                # write PSUM at partition 64 (col tiling), so both heads land
                # directly on their block-diag partitions - no DMA hop.
                tp = psA.tile([P, D], F32, tag="tp", name=f"tpa{b}_{h}",
                              space="PSUM")
                nc.tensor.matmul(tp[rb:rb + D, 0:D], aex[rb:rb + D, :],
                                 ident[rb:rb + D, rb:rb + D],
                                 start=True, stop=True)
                nc.vector.tensor_copy(a2[rb:rb + D, rb:rb + D],
                                      tp[rb:rb + D, 0:D])
            # T for both heads of the pair: one K=128 matmul per 512 cols
            for nf in range(2):
                ps = psB.tile([P, 512], F32, tag="psB", name=f"psT{b}_{pr}_{nf}",
                              space="PSUM")
                nc.tensor.matmul(ps[:], a2[:],
                                 qT[pr][:, nf * 512:(nf + 1) * 512],
                                 start=True, stop=True)
                nc.vector.tensor_copy(outT[pr][:, nf * 512:(nf + 1) * 512],
                                      ps[:])

        # ---- Phase D: y = out @ w_out ----
        for m in range(NM):
            yt = y_pool.tile([P, C], F32, tag="y", name=f"y{b}_{m}")
            for f in range(2):
                ps = psB.tile([P, 384], F32, tag="psB", name=f"psy{b}_{m}_{f}",
                              space="PSUM")
                for p in range(KC):
                    nc.tensor.matmul(
                        ps[:],
                        outT[p][:, m * P:(m + 1) * P],
                        wo[p][:, f * 384:(f + 1) * 384],
                        start=(p == 0), stop=(p == KC - 1))
                nc.vector.tensor_copy(yt[:, f * 384:(f + 1) * 384], ps[:])
            nc.sync.dma_start(y_d[b, m * P:(m + 1) * P, :], yt[:])


_BUILD_CACHE = {}


def build_program(mm_dt_name=MM_DT_NAME):
    if mm_dt_name in _BUILD_CACHE:
        return _BUILD_CACHE[mm_dt_name]
    mm_dt = F32R if mm_dt_name == "f32r" else F32
    nc = bacc.Bacc("TRN2", target_bir_lowering=False, debug=False,
                   num_devices=NCORES)
    x_d = nc.dram_tensor("x", [BS, N, C], F32, kind="ExternalInput").ap()
    wqkv_d = nc.dram_tensor("w_qkv", [C, 3 * C], F32, kind="ExternalInput").ap()
    wo_d = nc.dram_tensor("w_out", [C, C], F32, kind="ExternalInput").ap()
    y_d = nc.dram_tensor("y", [BS, N, C], F32, kind="ExternalOutput").ap()
    with tile.TileContext(nc) as tc:
        with ExitStack() as ctx:
            _emit(ctx, tc, mm_dt, x_d, wqkv_d, wo_d, y_d)
    nc.compile()
    _BUILD_CACHE[mm_dt_name] = nc
    return nc


def make_in_maps(x, w_qkv, w_out):
    x = np.ascontiguousarray(np.asarray(x, dtype=np.float32))
    w_qkv = np.ascontiguousarray(np.asarray(w_qkv, dtype=np.float32))
    w_out = np.ascontiguousarray(np.asarray(w_out, dtype=np.float32))
    return [
        {"x": x[i * BS:(i + 1) * BS], "w_qkv": w_qkv, "w_out": w_out}
        for i in range(NCORES)
    ]


def kernel(x, w_qkv, b_qkv=None, w_out=None, b_out=None, **_unused):
    nc = build_program()
    in_maps = make_in_maps(x, w_qkv, w_out)
    res = bass_utils.run_bass_kernel_spmd(nc, in_maps,
                                          core_ids=list(range(NCORES)))
    y = np.concatenate([res.results[i]["y"] for i in range(NCORES)], axis=0)
    return np.asarray(y, dtype=np.float32)


# revision 21
# speedup vs baseline: 1.0479x; 1.0001x over previous
"""ChannelMHSA on Trainium2 (Bass/Tile), data-parallel over batch on 8 cores.

Reference computation (per batch b of x [N, C]):
    qkv  = x @ w_qkv                      # [N, 3C], columns ordered (s, h, d)
    q, k, v per head h: [N, D]
    z_h  = k_h^T @ v_h / sqrt(D)          # [D, D]
    A_h  = softmax(z_h, axis=-1)
    T_h  = A_h @ q_h^T                    # [D, N]
    out[n, h*D+d] = T_h[d, n]
    y    = out @ w_out                    # [N, C]

b_qkv / b_out are all-zero by construction (see input spec) and are ignored.

Kernel layout choices per core (BS=4 batches):
  - xT [C, N] built by PE transposes (6x8 [128,128] blocks per batch).
  - kv = x @ w_qkv[:, C:3C] computed N-major (lhsT = xT chunks).
  - qT = w_q^T @ x^T computed C-major directly (lhsT = w_q chunks,
    rhs = xT chunks), so q never needs a separate transpose.
  - z per head with rhs packed 4 heads wide (free=256) for PE efficiency.
  - A^T placed into a block-diagonal [128,128] lhsT per head pair so
    T for two heads is one K=128 matmul per 512 output columns.
  - y = out @ w_out with lhsT = outT chunks.
"""

import os
import sys
from contextlib import ExitStack

import numpy as np

for _p in ("/opt/trn_rl_repo", "/opt/pypackages"):
    if _p not in sys.path:
        sys.path.append(_p)

import concourse.bacc as bacc
import concourse.mybir as mybir
import concourse.tile as tile
from concourse import bass_utils, masks

B, N, C = 32, 1024, 768
H, D = 12, 64
P = 128
NCORES = 8
BS = B // NCORES          # batches per core
KC = C // P               # 6 contraction chunks over C
NM = N // P               # 8 chunks over N
F32 = mybir.dt.float32
F32R = mybir.dt.float32r

# float32r runs the PE at 4x fp32 speed for free-dim >= 256 with slightly
# reduced mantissa precision. Override with BASS_MM_DT=f32 to compare.
MM_DT_NAME = os.environ.get("BASS_MM_DT", "f32r")


def _emit(ctx, tc, mm_dt, x_d, wqkv_d, wo_d, y_d):
    nc = tc.nc

    mdt = mm_dt          # dtype for tiles consumed by regular matmuls
    def wcast(ap):       # DRAM-side view for weight DMAs
        return ap.bitcast(mdt) if mdt is not F32 else ap

    const = ctx.enter_context(tc.tile_pool(name="const", bufs=1))
    xin_pool = ctx.enter_context(tc.tile_pool(name="xin", bufs=2))
    xt_pool = ctx.enter_context(tc.tile_pool(name="xtp", bufs=6))
    kv_pool = ctx.enter_context(tc.tile_pool(name="kvp", bufs=8))
    # qT and outT share slots: outT[pr] is produced right after the T matmul
    # of pair pr, which is also the last reader of qT[pr] - zero stall.
    qt_pool = ctx.enter_context(tc.tile_pool(name="qtp", bufs=6))
    y_pool = ctx.enter_context(tc.tile_pool(name="yp", bufs=2))
    sm_pool = ctx.enter_context(tc.tile_pool(name="smp", bufs=4))
    psA = ctx.enter_context(tc.tile_pool(name="psA", bufs=2, space="PSUM"))
    psB = ctx.enter_context(tc.tile_pool(name="psB", bufs=3, space="PSUM"))
    psZ = ctx.enter_context(tc.tile_pool(name="psZ", bufs=3, space="PSUM"))

    ident = const.tile([P, P], F32, tag="ident", name="ident")
    masks.make_identity(nc, ident[:])

    # Two persistent block-diag lhsT tiles for the T matmul, zeroed once via
    # a rounding copy (memset cannot produce float32r). Only the diagonal
    # blocks are rewritten afterwards, so the off-diag zeros persist.
    zeros = const.tile([P, P], F32, tag="zeros", name="zeros")
    nc.vector.memset(zeros[:], 0.0)
    a2_tiles = []
    for i in range(2):
        a2t = const.tile([P, P], mdt, tag=f"a2_{i}", name=f"a2_{i}")
        nc.vector.tensor_copy(a2t[:], zeros[:])
        a2_tiles.append(a2t)

    # Weight loads go on the Activation HWDGE queue so they don't delay the
    # first x chunks (Sync queue) that gate the first PE transposes. Order
    # matters: wq first (gates the qT phase, which is emitted before kv),
    # then wkv, then wo.
    wq = []
    for p in range(KC):
        t = const.tile([P, C], mdt, tag=f"wq{p}", name=f"wq{p}")
        nc.scalar.dma_start(t[:], wcast(wqkv_d[p * P:(p + 1) * P, 0:C]))
        wq.append(t)
    wkv = []
    for p in range(KC):
        t = const.tile([P, 2 * C], mdt, tag=f"wkv{p}", name=f"wkv{p}")
        nc.scalar.dma_start(t[:], wcast(wqkv_d[p * P:(p + 1) * P, C:3 * C]))
        wkv.append(t)
    wo = []
    for p in range(KC):
        t = const.tile([P, C], mdt, tag=f"wo{p}", name=f"wo{p}")
        nc.scalar.dma_start(t[:], wcast(wo_d[p * P:(p + 1) * P, :]))
        wo.append(t)

    for b in range(BS):
        # ---- Phase A: load x, transpose to xT [C, N] ----
        xT = [xt_pool.tile([P, N], mdt, tag="xT", name=f"xT{b}_{p}")
              for p in range(KC)]
        for m in range(NM):
            xin = xin_pool.tile([P, C], F32, tag="xin", name=f"xin{b}_{m}")
            nc.sync.dma_start(xin[:], x_d[b, m * P:(m + 1) * P, :])
            for p in range(KC):
                tp = psA.tile([P, P], F32, tag="tp", name=f"tpx{b}_{m}_{p}",
                              space="PSUM")
                nc.tensor.transpose(tp[:], xin[:, p * P:(p + 1) * P], ident[:])
                nc.vector.tensor_copy(xT[p][:, m * P:(m + 1) * P], tp[:])

        # ---- Phase B1: qT = w_q^T @ x^T, C-major (w_q lands first) ----
        qT = []
        for po in range(KC):
            qtt = qt_pool.tile([P, N], mdt, tag="qT", name=f"qT{b}_{po}")
            qT.append(qtt)
            for nf in range(2):
                ps = psB.tile([P, 512], F32, tag="psB", name=f"psqt{b}_{po}_{nf}",
                              space="PSUM")
                for p in range(KC):
                    nc.tensor.matmul(
                        ps[:],
                        wq[p][:, po * P:(po + 1) * P],
                        xT[p][:, nf * 512:(nf + 1) * 512],
                        start=(p == 0), stop=(p == KC - 1))
                nc.vector.tensor_copy(qtt[:, nf * 512:(nf + 1) * 512], ps[:])

        # ---- Phase B2: kv = x @ w_qkv[:, C:3C], N-major ----
        kv = []
        for m in range(NM):
            kvt = kv_pool.tile([P, 2 * C], mdt, tag="kv", name=f"kv{b}_{m}")
            kv.append(kvt)
            for f in range(3):
                ps = psB.tile([P, 512], F32, tag="psB", name=f"pskv{b}_{m}_{f}",
                              space="PSUM")
                for p in range(KC):
                    nc.tensor.matmul(
                        ps[:],
                        xT[p][:, m * P:(m + 1) * P],
                        wkv[p][:, f * 512:(f + 1) * 512],
                        start=(p == 0), stop=(p == KC - 1))
                nc.vector.tensor_copy(kvt[:, f * 512:(f + 1) * 512], ps[:])

        # ---- Phase C: attention, software-pipelined by one head pair so the
        # next pair's z matmuls fill the PE while this pair's softmax runs on
        # DVE/ACT. ----
        outT = [qt_pool.tile([P, N], mdt, tag="qT", name=f"outT{b}_{p}")
                for p in range(KC)]
        # Softmax needs no max-subtraction here: |z/8| <= ~25 so exp() is
        # fp32-safe, and softmax is shift-invariant. The 1/sum normalization
        # is deferred into the outT copy (per-partition scalar), so the only
        # serial op between z and the A^T matmul is the exp itself. z chains
        # are emitted LOOKAHEAD pairs ahead to keep the PE fed while exp runs.
        LOOKAHEAD = 2
        zps_pair = {}
        for step in range(KC + LOOKAHEAD):
            if step < KC:
                pr, q4 = step, step // 2
                # z for both heads of the pair in one chain: lhsT packs the
                # two heads' k (M=128), rhs packs 4 heads of v (free=256).
                # Head 2pr lands on psum rows 0:64, head 2pr+1 on 64:128.
                zps = psZ.tile([P, 256], F32, tag="z", name=f"z{b}_{pr}",
                               space="PSUM")
                zps_pair[pr] = zps
                for m in range(NM):
                    nc.tensor.matmul(
                        zps[:],
                        kv[m][:, 2 * pr * D:(2 * pr + 2) * D],
                        kv[m][:, C + q4 * 256:C + (q4 + 1) * 256],
                        start=(m == 0), stop=(m == NM - 1))
            if step < LOOKAHEAD:
                continue
            pr = step - LOOKAHEAD
            a2 = a2_tiles[pr % 2]
            zps = zps_pair.pop(pr)
            ssum = sm_pool.tile([P, 1], F32, tag="ssum", name=f"ss{b}_{pr}")
            for j in range(2):
                h = 2 * pr + j
                rb = j * D                  # psum row base for this head
                cb = (h % 4) * D
                zsl = zps[rb:rb + D, cb:cb + D]
                aex = sm_pool.tile([P, D], F32, tag="aex", name=f"aex{b}_{h}")
                nc.scalar.activation(aex[rb:rb + D, :], zsl,
                                     mybir.ActivationFunctionType.Exp,
                                     bias=0.0, scale=0.125,
                                     accum_out=ssum[rb:rb + D, :])
                # A^T into block-diag slot j of a2 via a REGULAR matmul
                # (aex^T @ I). Unlike transpose-mode, a regular matmul may
                # write PSUM at partition 64 (col tiling), so both heads land
                # directly on their block-diag partitions - no DMA hop.
                tp = psA.tile([P, D], F32, tag="tp", name=f"tpa{b}_{h}",
                              space="PSUM")
                nc.tensor.matmul(tp[rb:rb + D, 0:D], aex[rb:rb + D, :],
                                 ident[rb:rb + D, rb:rb + D],
                                 start=True, stop=True)
                nc.vector.tensor_copy(a2[rb:rb + D, rb:rb + D],
                                      tp[rb:rb + D, 0:D])
            rinv = sm_pool.tile([P, 1], F32, tag="rinv", name=f"ri{b}_{pr}")
            nc.vector.reciprocal(rinv[:], ssum[:])
            # T for both heads of the pair: one K=128 matmul per 512 cols;
            # the copy out applies the deferred softmax normalization (rows
            # of T are head-dims d, matching rinv's partition layout).
            for nf in range(2):
                ps = psB.tile([P, 512], F32, tag="psB", name=f"psT{b}_{pr}_{nf}",
                              space="PSUM")
                nc.tensor.matmul(ps[:], a2[:],
                                 qT[pr][:, nf * 512:(nf + 1) * 512],
                                 start=True, stop=True)
                nc.vector.tensor_scalar_mul(outT[pr][:, nf * 512:(nf + 1) * 512],
                                            ps[:], rinv[:])

        # ---- Phase D: y = out @ w_out ----
        for m in range(NM):
            yt = y_pool.tile([P, C], F32, tag="y", name=f"y{b}_{m}")
            for f in range(2):
                ps = psB.tile([P, 384], F32, tag="psB", name=f"psy{b}_{m}_{f}",
                              space="PSUM")
                for p in range(KC):
                    nc.tensor.matmul(
                        ps[:],
                        outT[p][:, m * P:(m + 1) * P],
                        wo[p][:, f * 384:(f + 1) * 384],
                        start=(p == 0), stop=(p == KC - 1))
                nc.vector.tensor_copy(yt[:, f * 384:(f + 1) * 384], ps[:])
            nc.sync.dma_start(y_d[b, m * P:(m + 1) * P, :], yt[:])


_BUILD_CACHE = {}


def build_program(mm_dt_name=MM_DT_NAME):
    if mm_dt_name in _BUILD_CACHE:
        return _BUILD_CACHE[mm_dt_name]
    mm_dt = F32R if mm_dt_name == "f32r" else F32
    nc = bacc.Bacc("TRN2", target_bir_lowering=False, debug=False,
                   num_devices=NCORES)
    x_d = nc.dram_tensor("x", [BS, N, C], F32, kind="ExternalInput").ap()
    wqkv_d = nc.dram_tensor("w_qkv", [C, 3 * C], F32, kind="ExternalInput").ap()
    wo_d = nc.dram_tensor("w_out", [C, C], F32, kind="ExternalInput").ap()
    y_d = nc.dram_tensor("y", [BS, N, C], F32, kind="ExternalOutput").ap()
    with tile.TileContext(nc) as tc:
        with ExitStack() as ctx:
            _emit(ctx, tc, mm_dt, x_d, wqkv_d, wo_d, y_d)
    nc.compile()
    _BUILD_CACHE[mm_dt_name] = nc
    return nc


def make_in_maps(x, w_qkv, w_out):
    x = np.ascontiguousarray(np.asarray(x, dtype=np.float32))
    w_qkv = np.ascontiguousarray(np.asarray(w_qkv, dtype=np.float32))
    w_out = np.ascontiguousarray(np.asarray(w_out, dtype=np.float32))
    return [
        {"x": x[i * BS:(i + 1) * BS], "w_qkv": w_qkv, "w_out": w_out}
        for i in range(NCORES)
    ]


def kernel(x, w_qkv, b_qkv=None, w_out=None, b_out=None, **_unused):
    nc = build_program()
    in_maps = make_in_maps(x, w_qkv, w_out)
    res = bass_utils.run_bass_kernel_spmd(nc, in_maps,
                                          core_ids=list(range(NCORES)))
    y = np.concatenate([res.results[i]["y"] for i in range(NCORES)], axis=0)
    return np.asarray(y, dtype=np.float32)


# revision 24
# speedup vs baseline: 1.1039x; 1.0534x over previous
"""ChannelMHSA on Trainium2 (Bass/Tile), data-parallel over batch on 8 cores.

Reference computation (per batch b of x [N, C]):
    qkv  = x @ w_qkv                      # [N, 3C], columns ordered (s, h, d)
    q, k, v per head h: [N, D]
    z_h  = k_h^T @ v_h / sqrt(D)          # [D, D]
    A_h  = softmax(z_h, axis=-1)
    T_h  = A_h @ q_h^T                    # [D, N]
    out[n, h*D+d] = T_h[d, n]
    y    = out @ w_out                    # [N, C]

b_qkv / b_out are all-zero by construction (see input spec) and are ignored.

Kernel layout choices per core (BS=4 batches):
  - xT [C, N] built by PE transposes (6x8 [128,128] blocks per batch).
  - kv = x @ w_qkv[:, C:3C] computed N-major (lhsT = xT chunks).
  - qT = w_q^T @ x^T computed C-major directly (lhsT = w_q chunks,
    rhs = xT chunks), so q never needs a separate transpose.
  - z per head with rhs packed 4 heads wide (free=256) for PE efficiency.
  - A^T placed into a block-diagonal [128,128] lhsT per head pair so
    T for two heads is one K=128 matmul per 512 output columns.
  - y = out @ w_out with lhsT = outT chunks.
"""

import os
import sys
from contextlib import ExitStack

import numpy as np

for _p in ("/opt/trn_rl_repo", "/opt/pypackages"):
    if _p not in sys.path:
        sys.path.append(_p)

import concourse.bacc as bacc
import concourse.mybir as mybir
import concourse.tile as tile
from concourse import bass_utils, masks

B, N, C = 32, 1024, 768
H, D = 12, 64
P = 128
NCORES = 8
BS = B // NCORES          # batches per core
KC = C // P               # 6 contraction chunks over C
NM = N // P               # 8 chunks over N
F32 = mybir.dt.float32
F32R = mybir.dt.float32r

# float32r runs the PE at 4x fp32 speed for free-dim >= 256 with slightly
# reduced mantissa precision. Override with BASS_MM_DT=f32 to compare.
MM_DT_NAME = os.environ.get("BASS_MM_DT", "f32r")


def _emit(ctx, tc, mm_dt, x_d, wqkv_d, wo_d, y_d):
    nc = tc.nc

    mdt = mm_dt          # dtype for tiles consumed by regular matmuls
    def wcast(ap):       # DRAM-side view for weight DMAs
        return ap.bitcast(mdt) if mdt is not F32 else ap

    const = ctx.enter_context(tc.tile_pool(name="const", bufs=1))
    xin_pool = ctx.enter_context(tc.tile_pool(name="xin", bufs=2))
    xt_pool = ctx.enter_context(tc.tile_pool(name="xtp", bufs=6))
    kv_pool = ctx.enter_context(tc.tile_pool(name="kvp", bufs=8))
    # qT and outT share slots: outT[pr] is produced right after the T matmul
    # of pair pr, which is also the last reader of qT[pr] - zero stall.
    qt_pool = ctx.enter_context(tc.tile_pool(name="qtp", bufs=6))
    y_pool = ctx.enter_context(tc.tile_pool(name="yp", bufs=2))
    sm_pool = ctx.enter_context(tc.tile_pool(name="smp", bufs=4))
    psA = ctx.enter_context(tc.tile_pool(name="psA", bufs=3, space="PSUM"))
    psB = ctx.enter_context(tc.tile_pool(name="psB", bufs=3, space="PSUM"))
    psZ = ctx.enter_context(tc.tile_pool(name="psZ", bufs=2, space="PSUM"))

    ident = const.tile([P, P], F32, tag="ident", name="ident")
    masks.make_identity(nc, ident[:])

    # Two persistent block-diag lhsT tiles for the T matmul, zeroed once via
    # a rounding copy (memset cannot produce float32r). Only the diagonal
    # blocks are rewritten afterwards, so the off-diag zeros persist.
    zeros = const.tile([P, P], F32, tag="zeros", name="zeros")
    nc.vector.memset(zeros[:], 0.0)
    a2_tiles = []
    for i in range(2):
        a2t = const.tile([P, P], mdt, tag=f"a2_{i}", name=f"a2_{i}")
        nc.vector.tensor_copy(a2t[:], zeros[:])
        a2_tiles.append(a2t)

    # Weight loads go on the Activation HWDGE queue so they don't delay the
    # first x chunks (Sync queue) that gate the first PE transposes. Order
    # matters: wq first (gates the qT phase, which is emitted before kv),
    # then wkv, then wo.
    wq = []
    for p in range(KC):
        t = const.tile([P, C], mdt, tag=f"wq{p}", name=f"wq{p}")
        nc.scalar.dma_start(t[:], wcast(wqkv_d[p * P:(p + 1) * P, 0:C]))
        wq.append(t)
    wkv = []
    for p in range(KC):
        t = const.tile([P, 2 * C], mdt, tag=f"wkv{p}", name=f"wkv{p}")
        nc.scalar.dma_start(t[:], wcast(wqkv_d[p * P:(p + 1) * P, C:3 * C]))
        wkv.append(t)
    wo = []
    for p in range(KC):
        t = const.tile([P, C], mdt, tag=f"wo{p}", name=f"wo{p}")
        nc.scalar.dma_start(t[:], wcast(wo_d[p * P:(p + 1) * P, :]))
        wo.append(t)

    for b in range(BS):
        # ---- Phase A: load x, transpose to xT [C, N] ----
        xT = [xt_pool.tile([P, N], mdt, tag="xT", name=f"xT{b}_{p}")
              for p in range(KC)]
        for m in range(NM):
            xin = xin_pool.tile([P, C], F32, tag="xin", name=f"xin{b}_{m}")
            nc.sync.dma_start(xin[:], x_d[b, m * P:(m + 1) * P, :])
            for p in range(KC):
                tp = psA.tile([P, P], F32, tag="tp", name=f"tpx{b}_{m}_{p}",
                              space="PSUM")
                nc.tensor.transpose(tp[:], xin[:, p * P:(p + 1) * P], ident[:])
                # copy on ACT (mostly idle) so the loaded DVE never gates
                # the transpose pipeline via psA slot reuse
                nc.scalar.copy(xT[p][:, m * P:(m + 1) * P], tp[:])

        # ---- Phase B1: qT = w_q^T @ x^T, C-major (w_q lands first) ----
        qT = []
        for po in range(KC):
            qtt = qt_pool.tile([P, N], mdt, tag="qT", name=f"qT{b}_{po}")
            qT.append(qtt)
            for nf in range(2):
                ps = psB.tile([P, 512], F32, tag="psB", name=f"psqt{b}_{po}_{nf}",
                              space="PSUM")
                for p in range(KC):
                    nc.tensor.matmul(
                        ps[:],
                        wq[p][:, po * P:(po + 1) * P],
                        xT[p][:, nf * 512:(nf + 1) * 512],
                        start=(p == 0), stop=(p == KC - 1))
                nc.vector.tensor_copy(qtt[:, nf * 512:(nf + 1) * 512], ps[:])

        # ---- Phase B2: kv = x @ w_qkv[:, C:3C], N-major ----
        kv = []
        for m in range(NM):
            kvt = kv_pool.tile([P, 2 * C], mdt, tag="kv", name=f"kv{b}_{m}")
            kv.append(kvt)
            for f in range(3):
                ps = psB.tile([P, 512], F32, tag="psB", name=f"pskv{b}_{m}_{f}",
                              space="PSUM")
                for p in range(KC):
                    nc.tensor.matmul(
                        ps[:],
                        xT[p][:, m * P:(m + 1) * P],
                        wkv[p][:, f * 512:(f + 1) * 512],
                        start=(p == 0), stop=(p == KC - 1))
                nc.vector.tensor_copy(kvt[:, f * 512:(f + 1) * 512], ps[:])

        # ---- Phase C: attention, software-pipelined by one head pair so the
        # next pair's z matmuls fill the PE while this pair's softmax runs on
        # DVE/ACT. ----
        outT = [qt_pool.tile([P, N], mdt, tag="qT", name=f"outT{b}_{p}")
                for p in range(KC)]
        # Softmax needs no max-subtraction here: |z/8| <= ~25 so exp() is
        # fp32-safe, and softmax is shift-invariant. The 1/sum normalization
        # is deferred into the outT copy (per-partition scalar), so the only
        # serial op between z and the A^T matmul is the exp itself. z chains
        # are emitted LOOKAHEAD pairs ahead to keep the PE fed while exp runs.
        LOOKAHEAD = 1
        zps_pair = {}
        for step in range(KC + LOOKAHEAD):
            if step < KC:
                pr, q4 = step, step // 2
                # z for both heads of the pair in one chain: lhsT packs the
                # two heads' k (M=128), rhs packs 4 heads of v (free=256).
                # Head 2pr lands on psum rows 0:64, head 2pr+1 on 64:128.
                zps = psZ.tile([P, 256], F32, tag="z", name=f"z{b}_{pr}",
                               space="PSUM")
                zps_pair[pr] = zps
                for m in range(NM):
                    nc.tensor.matmul(
                        zps[:],
                        kv[m][:, 2 * pr * D:(2 * pr + 2) * D],
                        kv[m][:, C + q4 * 256:C + (q4 + 1) * 256],
                        start=(m == 0), stop=(m == NM - 1))
            if step < LOOKAHEAD:
                continue
            pr = step - LOOKAHEAD
            a2 = a2_tiles[pr % 2]
            zps = zps_pair.pop(pr)
            ssum = sm_pool.tile([P, 1], F32, tag="ssum", name=f"ss{b}_{pr}")
            for j in range(2):
                h = 2 * pr + j
                rb = j * D                  # psum row base for this head
                cb = (h % 4) * D
                zsl = zps[rb:rb + D, cb:cb + D]
                aex = sm_pool.tile([P, D], F32, tag="aex", name=f"aex{b}_{h}")
                nc.scalar.activation(aex[rb:rb + D, :], zsl,
                                     mybir.ActivationFunctionType.Exp,
                                     bias=0.0, scale=0.125,
                                     accum_out=ssum[rb:rb + D, :])
                # A^T into block-diag slot j of a2 via a REGULAR matmul
                # (aex^T @ I). Unlike transpose-mode, a regular matmul may
                # write PSUM at partition 64 (col tiling), so both heads land
                # directly on their block-diag partitions - no DMA hop.
                tp = psA.tile([P, D], F32, tag="tp", name=f"tpa{b}_{h}",
                              space="PSUM")
                nc.tensor.matmul(tp[rb:rb + D, 0:D], aex[rb:rb + D, :],
                                 ident[rb:rb + D, rb:rb + D],
                                 start=True, stop=True)
                nc.vector.tensor_copy(a2[rb:rb + D, rb:rb + D],
                                      tp[rb:rb + D, 0:D])
            rinv = sm_pool.tile([P, 1], F32, tag="rinv", name=f"ri{b}_{pr}")
            nc.vector.reciprocal(rinv[:], ssum[:])
            # T for both heads of the pair: one K=128 matmul per 512 cols;
            # the copy out applies the deferred softmax normalization (rows
            # of T are head-dims d, matching rinv's partition layout).
            for nf in range(2):
                ps = psB.tile([P, 512], F32, tag="psB", name=f"psT{b}_{pr}_{nf}",
                              space="PSUM")
                nc.tensor.matmul(ps[:], a2[:],
                                 qT[pr][:, nf * 512:(nf + 1) * 512],
                                 start=True, stop=True)
                nc.vector.tensor_scalar_mul(outT[pr][:, nf * 512:(nf + 1) * 512],
                                            ps[:], rinv[:])

        # ---- Phase D: y = out @ w_out ----
        for m in range(NM):
            yt = y_pool.tile([P, C], F32, tag="y", name=f"y{b}_{m}")
            for f in range(2):
                ps = psB.tile([P, 384], F32, tag="psB", name=f"psy{b}_{m}_{f}",
                              space="PSUM")
                for p in range(KC):
                    nc.tensor.matmul(
                        ps[:],
                        outT[p][:, m * P:(m + 1) * P],
                        wo[p][:, f * 384:(f + 1) * 384],
                        start=(p == 0), stop=(p == KC - 1))
                nc.vector.tensor_copy(yt[:, f * 384:(f + 1) * 384], ps[:])
            nc.sync.dma_start(y_d[b, m * P:(m + 1) * P, :], yt[:])


_BUILD_CACHE = {}


def build_program(mm_dt_name=MM_DT_NAME):
    if mm_dt_name in _BUILD_CACHE:
        return _BUILD_CACHE[mm_dt_name]
    mm_dt = F32R if mm_dt_name == "f32r" else F32
    nc = bacc.Bacc("TRN2", target_bir_lowering=False, debug=False,
                   num_devices=NCORES)
    x_d = nc.dram_tensor("x", [BS, N, C], F32, kind="ExternalInput").ap()
    wqkv_d = nc.dram_tensor("w_qkv", [C, 3 * C], F32, kind="ExternalInput").ap()
    wo_d = nc.dram_tensor("w_out", [C, C], F32, kind="ExternalInput").ap()
    y_d = nc.dram_tensor("y", [BS, N, C], F32, kind="ExternalOutput").ap()
    with tile.TileContext(nc) as tc:
        with ExitStack() as ctx:
            _emit(ctx, tc, mm_dt, x_d, wqkv_d, wo_d, y_d)
    nc.compile()
    _BUILD_CACHE[mm_dt_name] = nc
    return nc


def make_in_maps(x, w_qkv, w_out):
    x = np.ascontiguousarray(np.asarray(x, dtype=np.float32))
    w_qkv = np.ascontiguousarray(np.asarray(w_qkv, dtype=np.float32))
    w_out = np.ascontiguousarray(np.asarray(w_out, dtype=np.float32))
    return [
        {"x": x[i * BS:(i + 1) * BS], "w_qkv": w_qkv, "w_out": w_out}
        for i in range(NCORES)
    ]


def kernel(x, w_qkv, b_qkv=None, w_out=None, b_out=None, **_unused):
    nc = build_program()
    in_maps = make_in_maps(x, w_qkv, w_out)
    res = bass_utils.run_bass_kernel_spmd(nc, in_maps,
                                          core_ids=list(range(NCORES)))
    y = np.concatenate([res.results[i]["y"] for i in range(NCORES)], axis=0)
    return np.asarray(y, dtype=np.float32)


# revision 25
# speedup vs baseline: 1.1504x; 1.0421x over previous
"""ChannelMHSA on Trainium2 (Bass/Tile), data-parallel over batch on 8 cores.

Reference computation (per batch b of x [N, C]):
    qkv  = x @ w_qkv                      # [N, 3C], columns ordered (s, h, d)
    q, k, v per head h: [N, D]
    z_h  = k_h^T @ v_h / sqrt(D)          # [D, D]
    A_h  = softmax(z_h, axis=-1)
    T_h  = A_h @ q_h^T                    # [D, N]
    out[n, h*D+d] = T_h[d, n]
    y    = out @ w_out                    # [N, C]

b_qkv / b_out are all-zero by construction (see input spec) and are ignored.

Kernel layout choices per core (BS=4 batches):
  - xT [C, N] built by PE transposes (6x8 [128,128] blocks per batch).
  - kv = x @ w_qkv[:, C:3C] computed N-major (lhsT = xT chunks).
  - qT = w_q^T @ x^T computed C-major directly (lhsT = w_q chunks,
    rhs = xT chunks), so q never needs a separate transpose.
  - z per head with rhs packed 4 heads wide (free=256) for PE efficiency.
  - A^T placed into a block-diagonal [128,128] lhsT per head pair so
    T for two heads is one K=128 matmul per 512 output columns.
  - y = out @ w_out with lhsT = outT chunks.
"""

import os
import sys
from contextlib import ExitStack

import numpy as np

for _p in ("/opt/trn_rl_repo", "/opt/pypackages"):
    if _p not in sys.path:
        sys.path.append(_p)

import concourse.bacc as bacc
import concourse.mybir as mybir
import concourse.tile as tile
from concourse import bass_utils, masks

B, N, C = 32, 1024, 768
H, D = 12, 64
P = 128
NCORES = 8
BS = B // NCORES          # batches per core
KC = C // P               # 6 contraction chunks over C
NM = N // P               # 8 chunks over N
F32 = mybir.dt.float32
F32R = mybir.dt.float32r

# float32r runs the PE at 4x fp32 speed for free-dim >= 256 with slightly
# reduced mantissa precision. Override with BASS_MM_DT=f32 to compare.
MM_DT_NAME = os.environ.get("BASS_MM_DT", "f32r")


def _emit(ctx, tc, mm_dt, x_d, wqkv_d, wo_d, y_d):
    nc = tc.nc

    mdt = mm_dt          # dtype for tiles consumed by regular matmuls
    def wcast(ap):       # DRAM-side view for weight DMAs
        return ap.bitcast(mdt) if mdt is not F32 else ap

    const = ctx.enter_context(tc.tile_pool(name="const", bufs=1))
    xin_pool = ctx.enter_context(tc.tile_pool(name="xin", bufs=4))
    xt_pool = ctx.enter_context(tc.tile_pool(name="xtp", bufs=6))
    kv_pool = ctx.enter_context(tc.tile_pool(name="kvp", bufs=8))
    # qT and outT share slots: outT[pr] is produced right after the T matmul
    # of pair pr, which is also the last reader of qT[pr] - zero stall.
    qt_pool = ctx.enter_context(tc.tile_pool(name="qtp", bufs=6))
    y_pool = ctx.enter_context(tc.tile_pool(name="yp", bufs=2))
    sm_pool = ctx.enter_context(tc.tile_pool(name="smp", bufs=4))
    psA = ctx.enter_context(tc.tile_pool(name="psA", bufs=3, space="PSUM"))
    psB = ctx.enter_context(tc.tile_pool(name="psB", bufs=3, space="PSUM"))
    psZ = ctx.enter_context(tc.tile_pool(name="psZ", bufs=2, space="PSUM"))

    ident = const.tile([P, P], F32, tag="ident", name="ident")
    masks.make_identity(nc, ident[:])

    # Two persistent block-diag lhsT tiles for the T matmul, zeroed once via
    # a rounding copy (memset cannot produce float32r). Only the diagonal
    # blocks are rewritten afterwards, so the off-diag zeros persist.
    zeros = const.tile([P, P], F32, tag="zeros", name="zeros")
    nc.vector.memset(zeros[:], 0.0)
    a2_tiles = []
    for i in range(2):
        a2t = const.tile([P, P], mdt, tag=f"a2_{i}", name=f"a2_{i}")
        nc.vector.tensor_copy(a2t[:], zeros[:])
        a2_tiles.append(a2t)

    # Weight loads go on the Activation HWDGE queue so they don't delay the
    # first x chunks (Sync queue) that gate the first PE transposes. Order
    # matters: wq first (gates the qT phase, which is emitted before kv),
    # then wkv, then wo.
    wq = []
    for p in range(KC):
        t = const.tile([P, C], mdt, tag=f"wq{p}", name=f"wq{p}")
        nc.sync.dma_start(t[:], wcast(wqkv_d[p * P:(p + 1) * P, 0:C]))
        wq.append(t)
    wkv = []
    for p in range(KC):
        t = const.tile([P, 2 * C], mdt, tag=f"wkv{p}", name=f"wkv{p}")
        nc.sync.dma_start(t[:], wcast(wqkv_d[p * P:(p + 1) * P, C:3 * C]))
        wkv.append(t)
    wo = []
    for p in range(KC):
        t = const.tile([P, C], mdt, tag=f"wo{p}", name=f"wo{p}")
        nc.sync.dma_start(t[:], wcast(wo_d[p * P:(p + 1) * P, :]))
        wo.append(t)

    def phase_a(b):
        # ---- Phase A: load x, transpose to xT [C, N] ----
        xT = [xt_pool.tile([P, N], mdt, tag="xT", name=f"xT{b}_{p}")
              for p in range(KC)]
        for m in range(NM):
            xin = xin_pool.tile([P, C], F32, tag="xin", name=f"xin{b}_{m}")
            nc.sync.dma_start(xin[:], x_d[b, m * P:(m + 1) * P, :])
            for p in range(KC):
                tp = psA.tile([P, P], F32, tag="tp", name=f"tpx{b}_{m}_{p}",
                              space="PSUM")
                nc.tensor.transpose(tp[:], xin[:, p * P:(p + 1) * P], ident[:])
                # copy on ACT (mostly idle) so the loaded DVE never gates
                # the transpose pipeline via psA slot reuse
                nc.scalar.copy(xT[p][:, m * P:(m + 1) * P], tp[:])
        return xT

    for b in range(BS):
        xT = phase_a(b)

        # ---- Phase B1: qT = w_q^T @ x^T, C-major (w_q lands first) ----
        qT = []
        for po in range(KC):
            qtt = qt_pool.tile([P, N], mdt, tag="qT", name=f"qT{b}_{po}")
            qT.append(qtt)
            for nf in range(2):
                ps = psB.tile([P, 512], F32, tag="psB", name=f"psqt{b}_{po}_{nf}",
                              space="PSUM")
                for p in range(KC):
                    nc.tensor.matmul(
                        ps[:],
                        wq[p][:, po * P:(po + 1) * P],
                        xT[p][:, nf * 512:(nf + 1) * 512],
                        start=(p == 0), stop=(p == KC - 1))
                nc.vector.tensor_copy(qtt[:, nf * 512:(nf + 1) * 512], ps[:])

        # ---- Phase B2: kv = x @ w_qkv[:, C:3C], N-major ----
        kv = []
        for m in range(NM):
            kvt = kv_pool.tile([P, 2 * C], mdt, tag="kv", name=f"kv{b}_{m}")
            kv.append(kvt)
            for f in range(3):
                ps = psB.tile([P, 512], F32, tag="psB", name=f"pskv{b}_{m}_{f}",
                              space="PSUM")
                for p in range(KC):
                    nc.tensor.matmul(
                        ps[:],
                        xT[p][:, m * P:(m + 1) * P],
                        wkv[p][:, f * 512:(f + 1) * 512],
                        start=(p == 0), stop=(p == KC - 1))
                nc.vector.tensor_copy(kvt[:, f * 512:(f + 1) * 512], ps[:])

        # ---- Phase C: attention, software-pipelined by one head pair so the
        # next pair's z matmuls fill the PE while this pair's softmax runs on
        # DVE/ACT. ----
        outT = [qt_pool.tile([P, N], mdt, tag="qT", name=f"outT{b}_{p}")
                for p in range(KC)]
        # Softmax needs no max-subtraction here: |z/8| <= ~25 so exp() is
        # fp32-safe, and softmax is shift-invariant. The 1/sum normalization
        # is deferred into the outT copy (per-partition scalar), so the only
        # serial op between z and the A^T matmul is the exp itself. z chains
        # are emitted LOOKAHEAD pairs ahead to keep the PE fed while exp runs.
        LOOKAHEAD = 1
        zps_pair = {}
        for step in range(KC + LOOKAHEAD):
            if step < KC:
                pr, q4 = step, step // 2
                # z for both heads of the pair in one chain: lhsT packs the
                # two heads' k (M=128), rhs packs 4 heads of v (free=256).
                # Head 2pr lands on psum rows 0:64, head 2pr+1 on 64:128.
                zps = psZ.tile([P, 256], F32, tag="z", name=f"z{b}_{pr}",
                               space="PSUM")
                zps_pair[pr] = zps
                for m in range(NM):
                    nc.tensor.matmul(
                        zps[:],
                        kv[m][:, 2 * pr * D:(2 * pr + 2) * D],
                        kv[m][:, C + q4 * 256:C + (q4 + 1) * 256],
                        start=(m == 0), stop=(m == NM - 1))
            if step < LOOKAHEAD:
                continue
            pr = step - LOOKAHEAD
            a2 = a2_tiles[pr % 2]
            zps = zps_pair.pop(pr)
            ssum = sm_pool.tile([P, 1], F32, tag="ssum", name=f"ss{b}_{pr}")
            for j in range(2):
                h = 2 * pr + j
                rb = j * D                  # psum row base for this head
                cb = (h % 4) * D
                zsl = zps[rb:rb + D, cb:cb + D]
                aex = sm_pool.tile([P, D], F32, tag="aex", name=f"aex{b}_{h}")
                nc.scalar.activation(aex[rb:rb + D, :], zsl,
                                     mybir.ActivationFunctionType.Exp,
                                     bias=0.0, scale=0.125,
                                     accum_out=ssum[rb:rb + D, :])
                # A^T into block-diag slot j of a2 via a REGULAR matmul
                # (aex^T @ I). Unlike transpose-mode, a regular matmul may
                # write PSUM at partition 64 (col tiling), so both heads land
                # directly on their block-diag partitions - no DMA hop.
                tp = psA.tile([P, D], F32, tag="tp", name=f"tpa{b}_{h}",
                              space="PSUM")
                nc.tensor.matmul(tp[rb:rb + D, 0:D], aex[rb:rb + D, :],
                                 ident[rb:rb + D, rb:rb + D],
                                 start=True, stop=True)
                nc.vector.tensor_copy(a2[rb:rb + D, rb:rb + D],
                                      tp[rb:rb + D, 0:D])
            rinv = sm_pool.tile([P, 1], F32, tag="rinv", name=f"ri{b}_{pr}")
            nc.vector.reciprocal(rinv[:], ssum[:])
            # T for both heads of the pair: one K=128 matmul per 512 cols;
            # the copy out applies the deferred softmax normalization (rows
            # of T are head-dims d, matching rinv's partition layout).
            for nf in range(2):
                ps = psB.tile([P, 512], F32, tag="psB", name=f"psT{b}_{pr}_{nf}",
                              space="PSUM")
                nc.tensor.matmul(ps[:], a2[:],
                                 qT[pr][:, nf * 512:(nf + 1) * 512],
                                 start=True, stop=True)
                nc.vector.tensor_scalar_mul(outT[pr][:, nf * 512:(nf + 1) * 512],
                                            ps[:], rinv[:])

        # ---- Phase D: y = out @ w_out ----
        for m in range(NM):
            yt = y_pool.tile([P, C], F32, tag="y", name=f"y{b}_{m}")
            for f in range(2):
                ps = psB.tile([P, 384], F32, tag="psB", name=f"psy{b}_{m}_{f}",
                              space="PSUM")
                for p in range(KC):
                    nc.tensor.matmul(
                        ps[:],
                        outT[p][:, m * P:(m + 1) * P],
                        wo[p][:, f * 384:(f + 1) * 384],
                        start=(p == 0), stop=(p == KC - 1))
                nc.vector.tensor_copy(yt[:, f * 384:(f + 1) * 384], ps[:])
            nc.sync.dma_start(y_d[b, m * P:(m + 1) * P, :], yt[:])


_BUILD_CACHE = {}


def build_program(mm_dt_name=MM_DT_NAME):
    if mm_dt_name in _BUILD_CACHE:
        return _BUILD_CACHE[mm_dt_name]
    mm_dt = F32R if mm_dt_name == "f32r" else F32
    nc = bacc.Bacc("TRN2", target_bir_lowering=False, debug=False,
                   num_devices=NCORES)
    x_d = nc.dram_tensor("x", [BS, N, C], F32, kind="ExternalInput").ap()
    wqkv_d = nc.dram_tensor("w_qkv", [C, 3 * C], F32, kind="ExternalInput").ap()
    wo_d = nc.dram_tensor("w_out", [C, C], F32, kind="ExternalInput").ap()
    y_d = nc.dram_tensor("y", [BS, N, C], F32, kind="ExternalOutput").ap()
    with tile.TileContext(nc) as tc:
        with ExitStack() as ctx:
            _emit(ctx, tc, mm_dt, x_d, wqkv_d, wo_d, y_d)
    nc.compile()
    _BUILD_CACHE[mm_dt_name] = nc
    return nc


def make_in_maps(x, w_qkv, w_out):
    x = np.ascontiguousarray(np.asarray(x, dtype=np.float32))
    w_qkv = np.ascontiguousarray(np.asarray(w_qkv, dtype=np.float32))
    w_out = np.ascontiguousarray(np.asarray(w_out, dtype=np.float32))
    return [
        {"x": x[i * BS:(i + 1) * BS], "w_qkv": w_qkv, "w_out": w_out}
        for i in range(NCORES)
    ]


def kernel(x, w_qkv, b_qkv=None, w_out=None, b_out=None, **_unused):
    nc = build_program()
    in_maps = make_in_maps(x, w_qkv, w_out)
    res = bass_utils.run_bass_kernel_spmd(nc, in_maps,
                                          core_ids=list(range(NCORES)))
    y = np.concatenate([res.results[i]["y"] for i in range(NCORES)], axis=0)
    return np.asarray(y, dtype=np.float32)


# revision 27
# speedup vs baseline: 1.2272x; 1.0668x over previous
"""ChannelMHSA on Trainium2 (Bass/Tile), data-parallel over batch on 8 cores.

Reference computation (per batch b of x [N, C]):
    qkv  = x @ w_qkv                      # [N, 3C], columns ordered (s, h, d)
    q, k, v per head h: [N, D]
    z_h  = k_h^T @ v_h / sqrt(D)          # [D, D]
    A_h  = softmax(z_h, axis=-1)
    T_h  = A_h @ q_h^T                    # [D, N]
    out[n, h*D+d] = T_h[d, n]
    y    = out @ w_out                    # [N, C]

b_qkv / b_out are all-zero by construction (see input spec) and are ignored.

Kernel layout choices per core (BS=4 batches):
  - xT [C, N] built by PE transposes (6x8 [128,128] blocks per batch).
  - kv = x @ w_qkv[:, C:3C] computed N-major (lhsT = xT chunks).
  - qT = w_q^T @ x^T computed C-major directly (lhsT = w_q chunks,
    rhs = xT chunks), so q never needs a separate transpose.
  - z per head with rhs packed 4 heads wide (free=256) for PE efficiency.
  - A^T placed into a block-diagonal [128,128] lhsT per head pair so
    T for two heads is one K=128 matmul per 512 output columns.
  - y = out @ w_out with lhsT = outT chunks.
"""

import os
import sys
from contextlib import ExitStack

import numpy as np

for _p in ("/opt/trn_rl_repo", "/opt/pypackages"):
    if _p not in sys.path:
        sys.path.append(_p)

import concourse.bacc as bacc
import concourse.mybir as mybir
import concourse.tile as tile
from concourse import bass_utils, masks

B, N, C = 32, 1024, 768
H, D = 12, 64
P = 128
NCORES = 8
BS = B // NCORES          # batches per core
KC = C // P               # 6 contraction chunks over C
NM = N // P               # 8 chunks over N
F32 = mybir.dt.float32
F32R = mybir.dt.float32r

# float32r runs the PE at 4x fp32 speed for free-dim >= 256 with slightly
# reduced mantissa precision. Override with BASS_MM_DT=f32 to compare.
MM_DT_NAME = os.environ.get("BASS_MM_DT", "f32r")


def _emit(ctx, tc, mm_dt, x_d, wqkv_d, wo_d, y_d):
    nc = tc.nc

    mdt = mm_dt          # dtype for tiles consumed by regular matmuls
    def wcast(ap):       # DRAM-side view for weight DMAs
        return ap.bitcast(mdt) if mdt is not F32 else ap

    const = ctx.enter_context(tc.tile_pool(name="const", bufs=1))
    xin_pool = ctx.enter_context(tc.tile_pool(name="xin", bufs=4))
    xt_pool = ctx.enter_context(tc.tile_pool(name="xtp", bufs=6))
    kv_pool = ctx.enter_context(tc.tile_pool(name="kvp", bufs=8))
    # qT and outT share slots: outT[pr] is produced right after the T matmul
    # of pair pr, which is also the last reader of qT[pr] - zero stall.
    qt_pool = ctx.enter_context(tc.tile_pool(name="qtp", bufs=6))
    y_pool = ctx.enter_context(tc.tile_pool(name="yp", bufs=2))
    sm_pool = ctx.enter_context(tc.tile_pool(name="smp", bufs=4))
    psA = ctx.enter_context(tc.tile_pool(name="psA", bufs=3, space="PSUM"))
    psB = ctx.enter_context(tc.tile_pool(name="psB", bufs=3, space="PSUM"))
    psZ = ctx.enter_context(tc.tile_pool(name="psZ", bufs=2, space="PSUM"))

    ident = const.tile([P, P], F32, tag="ident", name="ident")
    masks.make_identity(nc, ident[:])

    # Two persistent block-diag lhsT tiles for the T matmul, zeroed once via
    # a rounding copy (memset cannot produce float32r). Only the diagonal
    # blocks are rewritten afterwards, so the off-diag zeros persist.
    zeros = const.tile([P, P], F32, tag="zeros", name="zeros")
    nc.vector.memset(zeros[:], 0.0)
    a2_tiles = []
    for i in range(2):
        a2t = const.tile([P, P], mdt, tag=f"a2_{i}", name=f"a2_{i}")
        nc.vector.tensor_copy(a2t[:], zeros[:])
        a2_tiles.append(a2t)

    def phase_a(b):
        # ---- Phase A: load x, transpose to xT [C, N] ----
        xT = [xt_pool.tile([P, N], mdt, tag="xT", name=f"xT{b}_{p}")
              for p in range(KC)]
        for m in range(NM):
            xin = xin_pool.tile([P, C], F32, tag="xin", name=f"xin{b}_{m}")
            nc.sync.dma_start(xin[:], x_d[b, m * P:(m + 1) * P, :])
            for p in range(KC):
                tp = psA.tile([P, P], F32, tag="tp", name=f"tpx{b}_{m}_{p}",
                              space="PSUM")
                nc.tensor.transpose(tp[:], xin[:, p * P:(p + 1) * P], ident[:])
                # copy on ACT (mostly idle) so the loaded DVE never gates
                # the transpose pipeline via psA slot reuse
                nc.scalar.copy(xT[p][:, m * P:(m + 1) * P], tp[:])
        return xT

    # Batch-0 x chunks go on the Sync queue FIRST so the PE transposes start
    # immediately; the weight loads queue up behind them, ordered wq (gates
    # the qT phase) -> wkv -> wo.
    xT0 = phase_a(0)

    wq = []
    for p in range(KC):
        t = const.tile([P, C], mdt, tag=f"wq{p}", name=f"wq{p}")
        nc.sync.dma_start(t[:], wcast(wqkv_d[p * P:(p + 1) * P, 0:C]))
        wq.append(t)
    wkv = []
    for p in range(KC):
        t = const.tile([P, 2 * C], mdt, tag=f"wkv{p}", name=f"wkv{p}")
        nc.sync.dma_start(t[:], wcast(wqkv_d[p * P:(p + 1) * P, C:3 * C]))
        wkv.append(t)
    wo = []
    for p in range(KC):
        t = const.tile([P, C], mdt, tag=f"wo{p}", name=f"wo{p}")
        nc.sync.dma_start(t[:], wcast(wo_d[p * P:(p + 1) * P, :]))
        wo.append(t)

    for b in range(BS):
        xT = xT0 if b == 0 else phase_a(b)

        # ---- Phase B1: qT = w_q^T @ x^T, C-major (w_q lands first) ----
        qT = []
        for po in range(KC):
            qtt = qt_pool.tile([P, N], mdt, tag="qT", name=f"qT{b}_{po}")
            qT.append(qtt)
            for nf in range(2):
                ps = psB.tile([P, 512], F32, tag="psB", name=f"psqt{b}_{po}_{nf}",
                              space="PSUM")
                for p in range(KC):
                    nc.tensor.matmul(
                        ps[:],
                        wq[p][:, po * P:(po + 1) * P],
                        xT[p][:, nf * 512:(nf + 1) * 512],
                        start=(p == 0), stop=(p == KC - 1))
                nc.vector.tensor_copy(qtt[:, nf * 512:(nf + 1) * 512], ps[:])

        # ---- Phase B2: kv = x @ w_qkv[:, C:3C], N-major ----
        kv = []
        for m in range(NM):
            kvt = kv_pool.tile([P, 2 * C], mdt, tag="kv", name=f"kv{b}_{m}")
            kv.append(kvt)
            for f in range(3):
                ps = psB.tile([P, 512], F32, tag="psB", name=f"pskv{b}_{m}_{f}",
                              space="PSUM")
                for p in range(KC):
                    nc.tensor.matmul(
                        ps[:],
                        xT[p][:, m * P:(m + 1) * P],
                        wkv[p][:, f * 512:(f + 1) * 512],
                        start=(p == 0), stop=(p == KC - 1))
                nc.vector.tensor_copy(kvt[:, f * 512:(f + 1) * 512], ps[:])

        # ---- Phase C: attention, software-pipelined by one head pair so the
        # next pair's z matmuls fill the PE while this pair's softmax runs on
        # DVE/ACT. ----
        outT = [qt_pool.tile([P, N], mdt, tag="qT", name=f"outT{b}_{p}")
                for p in range(KC)]
        # Softmax needs no max-subtraction here: |z/8| <= ~25 so exp() is
        # fp32-safe, and softmax is shift-invariant. The 1/sum normalization
        # is deferred into the outT copy (per-partition scalar), so the only
        # serial op between z and the A^T matmul is the exp itself. z chains
        # are emitted LOOKAHEAD pairs ahead to keep the PE fed while exp runs.
        LOOKAHEAD = 1
        zps_pair = {}
        for step in range(KC + LOOKAHEAD):
            if step < KC:
                pr, q4 = step, step // 2
                # z for both heads of the pair in one chain: lhsT packs the
                # two heads' k (M=128), rhs packs 4 heads of v (free=256).
                # Head 2pr lands on psum rows 0:64, head 2pr+1 on 64:128.
                zps = psZ.tile([P, 256], F32, tag="z", name=f"z{b}_{pr}",
                               space="PSUM")
                zps_pair[pr] = zps
                for m in range(NM):
                    nc.tensor.matmul(
                        zps[:],
                        kv[m][:, 2 * pr * D:(2 * pr + 2) * D],
                        kv[m][:, C + q4 * 256:C + (q4 + 1) * 256],
                        start=(m == 0), stop=(m == NM - 1))
            if step < LOOKAHEAD:
                continue
            pr = step - LOOKAHEAD
            a2 = a2_tiles[pr % 2]
            zps = zps_pair.pop(pr)
            ssum = sm_pool.tile([P, 1], F32, tag="ssum", name=f"ss{b}_{pr}")
            for j in range(2):
                h = 2 * pr + j
                rb = j * D                  # psum row base for this head
                cb = (h % 4) * D
                zsl = zps[rb:rb + D, cb:cb + D]
                aex = sm_pool.tile([P, D], F32, tag="aex", name=f"aex{b}_{h}")
                nc.scalar.activation(aex[rb:rb + D, :], zsl,
                                     mybir.ActivationFunctionType.Exp,
                                     bias=0.0, scale=0.125,
                                     accum_out=ssum[rb:rb + D, :])
                # A^T into block-diag slot j of a2 via a REGULAR matmul
                # (aex^T @ I). Unlike transpose-mode, a regular matmul may
                # write PSUM at partition 64 (col tiling), so both heads land
                # directly on their block-diag partitions - no DMA hop.
                tp = psA.tile([P, D], F32, tag="tp", name=f"tpa{b}_{h}",
                              space="PSUM")
                nc.tensor.matmul(tp[rb:rb + D, 0:D], aex[rb:rb + D, :],
                                 ident[rb:rb + D, rb:rb + D],
                                 start=True, stop=True)
                nc.vector.tensor_copy(a2[rb:rb + D, rb:rb + D],
                                      tp[rb:rb + D, 0:D])
            rinv = sm_pool.tile([P, 1], F32, tag="rinv", name=f"ri{b}_{pr}")
            nc.vector.reciprocal(rinv[:], ssum[:])
            # T for both heads of the pair: one K=128 matmul per 512 cols;
            # the copy out applies the deferred softmax normalization (rows
            # of T are head-dims d, matching rinv's partition layout).
            for nf in range(2):
                ps = psB.tile([P, 512], F32, tag="psB", name=f"psT{b}_{pr}_{nf}",
                              space="PSUM")
                nc.tensor.matmul(ps[:], a2[:],
                                 qT[pr][:, nf * 512:(nf + 1) * 512],
                                 start=True, stop=True)
                nc.vector.tensor_scalar_mul(outT[pr][:, nf * 512:(nf + 1) * 512],
                                            ps[:], rinv[:])

        # ---- Phase D: y = out @ w_out ----
        for m in range(NM):
            yt = y_pool.tile([P, C], F32, tag="y", name=f"y{b}_{m}")
            for f in range(2):
                ps = psB.tile([P, 384], F32, tag="psB", name=f"psy{b}_{m}_{f}",
                              space="PSUM")
                for p in range(KC):
                    nc.tensor.matmul(
                        ps[:],
                        outT[p][:, m * P:(m + 1) * P],
                        wo[p][:, f * 384:(f + 1) * 384],
                        start=(p == 0), stop=(p == KC - 1))
                nc.vector.tensor_copy(yt[:, f * 384:(f + 1) * 384], ps[:])
            nc.sync.dma_start(y_d[b, m * P:(m + 1) * P, :], yt[:])


_BUILD_CACHE = {}


def build_program(mm_dt_name=MM_DT_NAME):
    if mm_dt_name in _BUILD_CACHE:
        return _BUILD_CACHE[mm_dt_name]
    mm_dt = F32R if mm_dt_name == "f32r" else F32
    nc = bacc.Bacc("TRN2", target_bir_lowering=False, debug=False,
                   num_devices=NCORES)
    x_d = nc.dram_tensor("x", [BS, N, C], F32, kind="ExternalInput").ap()
    wqkv_d = nc.dram_tensor("w_qkv", [C, 3 * C], F32, kind="ExternalInput").ap()
    wo_d = nc.dram_tensor("w_out", [C, C], F32, kind="ExternalInput").ap()
    y_d = nc.dram_tensor("y", [BS, N, C], F32, kind="ExternalOutput").ap()
    with tile.TileContext(nc) as tc:
        with ExitStack() as ctx:
            _emit(ctx, tc, mm_dt, x_d, wqkv_d, wo_d, y_d)
    nc.compile()
    _BUILD_CACHE[mm_dt_name] = nc
    return nc


def make_in_maps(x, w_qkv, w_out):
    x = np.ascontiguousarray(np.asarray(x, dtype=np.float32))
    w_qkv = np.ascontiguousarray(np.asarray(w_qkv, dtype=np.float32))
    w_out = np.ascontiguousarray(np.asarray(w_out, dtype=np.float32))
    return [
        {"x": x[i * BS:(i + 1) * BS], "w_qkv": w_qkv, "w_out": w_out}
        for i in range(NCORES)
    ]


def kernel(x, w_qkv, b_qkv=None, w_out=None, b_out=None, **_unused):
    nc = build_program()
    in_maps = make_in_maps(x, w_qkv, w_out)
    res = bass_utils.run_bass_kernel_spmd(nc, in_maps,
                                          core_ids=list(range(NCORES)))
    y = np.concatenate([res.results[i]["y"] for i in range(NCORES)], axis=0)
    return np.asarray(y, dtype=np.float32)
